# revision 19
# baseline (speedup 1.0000x reference)
"""Ternary-quantized 3x3 conv (stride 1, pad 1) on 8 trn2 NeuronCores.

Full inputs: X (32,128,56,56) f32, weight (256,128,3,3) f32, Wp/Wn (1,) f32.
Output: (32,256,56,56) f32.

Strategy: data-parallel over batch (4 images per core). Ternary weights are
exact in fp8e4; X is split into hi = fp8(x) and lo = fp8(x - hi). Per-core
implicit GEMM with fp8 DoubleRow matmuls (2 contraction k-tiles per
instruction at the fp16 streaming rate):
  - 5 "cross" taps (0,1),(1,0),(1,1),(1,2),(2,1): one DR matmul per tap with
    k-tiles (hi, lo) -> numerically exact to ~1e-3.
  - 4 corner taps: 2 DR matmuls, each pairing two taps at column offset +2 on
    the hi plane only (fp8-only error on 4/9 taps; measured rel err 1.7e-2).
7 matmuls per (image, oc-chunk, spatial tile) instead of 9 -> 1.29x on the
PE-bound phase. rhs windows are 464 contiguous elements (8 padded 58-wide
rows); the 2 junk columns per row are dropped during PSUM evacuation.
"""

import sys

sys.path.insert(0, "/opt/trn_rl_repo")

import numpy as np

import bass_rust
import concourse.bass as bass
import concourse.mybir as mybir
from concourse.ap import AP
from concourse.tile import TileContext
from concourse.bass_utils import run_bass_kernel_spmd

B, C_IN, C_OUT, KS, H, W = 32, 128, 256, 3, 56, 56
THRESHOLD = 0.05
N_CORES = 8
NPC = B // N_CORES  # images per core
WP_ = W + 2  # padded width (58)
HPAD = H + 3  # padded plane rows incl 1 slack row (59)
PLANE = HPAD * WP_  # k-tile stride between hi and lo planes (3422)
ROWS = 8  # output rows per spatial tile
NT = H // ROWS  # spatial tiles per image (7)
NFREE = ROWS * WP_  # matmul free dim incl junk cols (464)
NVALID = ROWS * W  # valid output elements per tile (448)
OCC = C_OUT // 128  # output channel chunks (2)

# tap schedule: corrected (hi+lo) taps and hi-only pairs
CORR = [(0, 1), (1, 0), (1, 1), (1, 2), (2, 1)]
PAIRS = [((0, 0), (0, 2)), ((2, 0), (2, 2))]  # delta = 2 columns
NW = len(CORR) + len(PAIRS)  # weight slots (7)

# walrus codegen in this container has tight per-instruction sync-wait
# encoding limits (DMA_DIRECT2D: 1, CTRL/Drain: <=2). Hoist excess waits onto
# preceding nop instructions on the same engine (safe: every non-Pool engine
# sequencer is a single strict-FIFO stream).
_MAX_WAITS = {
    "InstDMACopy": 1,
    "InstDrain": 1,
    "InstNop": 1,
    "InstNoOp": 1,
    "InstEventSemaphore": 1,
    "InstSemClear": 1,
}
_DEFAULT_MAX_WAITS = 1


def _split_ctrl_waits(nc, max_waits=None):
    for bbw in nc.main_func.blocks:
        il = bbw.instructions
        i = 0
        while i < len(il):
            ins = il[i]
            si = ins.sync_info
            if si is None or not si.on_wait:
                i += 1
                continue
            limit = _MAX_WAITS.get(type(ins).__name__, _DEFAULT_MAX_WAITS)
            if len(si.on_wait) > limit and str(ins.engine) != "EngineType.Pool":
                max_waits = limit
                waits = list(si.on_wait)
                keep, extra = waits[:max_waits], waits[max_waits:]
                new_insts = []
                for s in range(0, len(extra), max_waits):
                    chunk = extra[s : s + max_waits]
                    nop_ins = nc.engines[ins.engine].nop(nofuse=True).ins
                    for b2 in nc.main_func.blocks:
                        if b2.instructions and b2.instructions[-1] is nop_ins:
                            b2.instructions.pop()
                            break
                    nop_ins.sync_info = bass_rust.SyncInfo(
                        on_wait=chunk, on_update=[]
                    )
                    new_insts.append(nop_ins)
                si.on_wait = keep
                for k, nop_ins in enumerate(new_insts):
                    il.insert(i + k, nop_ins)
                i += len(new_insts)
            i += 1


def _build_nc():
    f32, f16, f8 = mybir.dt.float32, mybir.dt.float16, mybir.dt.float8e4
    nc = bass.Bass()
    x_in = nc.dram_tensor("X", [NPC, C_IN, H, W], f32, kind="ExternalInput")
    # k-tile pairs contiguous per oc chunk so LDWEIGHTS reads 256B strings
    w_in = nc.dram_tensor("W8", [C_IN, NW, OCC, 2, 128], f8, kind="ExternalInput")
    out = nc.dram_tensor("OUT", [NPC, C_OUT, H, W], f32, kind="ExternalOutput")

    DR = mybir.MatmulPerfMode.DoubleRow

    with TileContext(nc) as tc:
        with (
            tc.tile_pool(name="wp", bufs=1) as wp,
            tc.tile_pool(name="xs", bufs=3) as xsp,
            tc.tile_pool(name="xq", bufs=3) as xqp,
            tc.tile_pool(name="ps", bufs=8, space="PSUM") as psp,
            tc.tile_pool(name="ob", bufs=8) as obp,
        ):
            wt = wp.tile([C_IN, NW, OCC, 2, 128], f8)

            # PE warm-up: dummy matmuls keep TensorE busy through the
            # input-load phase so HAM is ramping toward K=8/8 (2.4 GHz) when
            # the real matmuls start.
            warm_sb = wp.tile([C_IN, 384], f16, name="warm_sb", tag="warm_sb")
            nc.vector.memset(warm_sb[:], 0.0)
            warm_ps = psp.tile([128, 256], f32, name="warm_ps", tag="warm", bufs=1)

            def warm(k):
                for _ in range(k):
                    nc.tensor.matmul(
                        warm_ps[:], warm_sb[:, 0:128], warm_sb[:, 128:384],
                        start=True, stop=True,
                    )

            warm(8)

            def make_xq(n):
                # [hi/lo plane, 59 rows (58 + slack), 58 cols] fp8
                xq = xqp.tile([C_IN, 2, HPAD, WP_], f8, name=f"xq_{n}", tag="xq")
                for k in range(2):
                    # top border row, bottom border + slack rows, side columns
                    nc.vector.memset(xq[:, k, 0, :], 0.0)
                    nc.vector.memset(xq[:, k, H + 1 : HPAD, :], 0.0)
                    nc.vector.memset(xq[:, k, 1 : H + 1, 0], 0.0)
                    nc.vector.memset(xq[:, k, 1 : H + 1, WP_ - 1], 0.0)
                return xq

            def load_chunk(xs, xq, n, r0, nrows):
                nc.sync.dma_start(
                    out=xs[:, r0 : r0 + nrows, :], in_=x_in[n, :, r0 : r0 + nrows, :]
                )
                hi = xq[:, 0, r0 + 1 : r0 + nrows + 1, 1 : WP_ - 1]
                nc.vector.tensor_copy(hi, xs[:, r0 : r0 + nrows, :])
                nc.vector.scalar_tensor_tensor(
                    out=xq[:, 1, r0 + 1 : r0 + nrows + 1, 1 : WP_ - 1],
                    in0=xs[:, r0 : r0 + nrows, :],
                    scalar=0.0,
                    in1=hi,
                    op0=mybir.AluOpType.bypass,
                    op1=mybir.AluOpType.subtract,
                )

            CH_STEADY = [(0, 14), (14, 14), (28, 14), (42, 14)]
            # image 0 uses small chunks so the first groups unblock quickly
            CH_FIRST = [(7 * k, 7) for k in range(8)]

            xs0 = xsp.tile([C_IN, H, W], f32, name="xs_0", tag="xs")
            xq0 = make_xq(0)
            # weights ride the idle scalar DMA queue so the first X chunk's
            # sync-queue issue isn't delayed behind the 460KB weight DMA
            nc.scalar.dma_start(out=wt[:], in_=w_in[:])
            for r0, nr in CH_FIRST:
                load_chunk(xs0, xq0, 0, r0, nr)

            def pair_rhs(xq, s, kh):
                # [part][(2 cols, 2)][(1, 464)] at hi-plane offset (s*8+kh)*58
                a = xq[:]
                return AP(
                    tensor=a.tensor,
                    offset=a.offset + (s * ROWS + kh) * WP_,
                    ap=[[2 * PLANE, C_IN], [2, 2], [1, NFREE]],
                )

            def corr_rhs(xq, s, kh, kw):
                a = xq[:]
                return AP(
                    tensor=a.tensor,
                    offset=a.offset + (s * ROWS + kh) * WP_ + kw,
                    ap=[[2 * PLANE, C_IN], [PLANE, 2], [1, NFREE]],
                )

            def matmul_group(xq, wtile, oc, s, ps):
                i = 0
                for ci, (kh, kw) in enumerate(CORR):
                    nc.tensor.matmul(
                        ps[:],
                        wtile[:, ci, oc, :, :],
                        corr_rhs(xq, s, kh, kw),
                        start=(i == 0),
                        stop=(i == NW - 1),
                        perf_mode=DR,
                    )
                    i += 1
                for pi, ((kh, kw1), _t2) in enumerate(PAIRS):
                    nc.tensor.matmul(
                        ps[:],
                        wtile[:, len(CORR) + pi, oc, :, :],
                        pair_rhs(xq, s, kh),
                        start=(i == 0),
                        stop=(i == NW - 1),
                        perf_mode=DR,
                    )
                    i += 1

            xs, xq = xs0, xq0
            # image 0: s-major across both oc chunks so early groups consume
            # input rows at half the rate the cast chain produces them
            n = 0
            for s in range(NT):
                if s == 2:
                    xs_next = xsp.tile([C_IN, H, W], f32, name="xs_1", tag="xs")
                    xq_next = make_xq(1)
                    for r0, nr in CH_STEADY:
                        load_chunk(xs_next, xq_next, 1, r0, nr)
                ps_a = psp.tile([128, ROWS, WP_], f32, tag="ps", name=f"ps_0a_{s}", bufs=7)
                matmul_group(xq, wt, 0, s, ps_a)
                if s <= 2:
                    warm(2)  # bridge input-chain stalls; keep HAM ramping
                ps_b = psp.tile([128, ROWS, WP_], f32, tag="ps", name=f"ps_0b_{s}", bufs=7)
                matmul_group(xq, wt, 1, s, ps_b)
                ob = obp.tile([128, 2 * NVALID], f32)
                nc.scalar.copy(ob[:, 0 : NVALID], ps_a[:, :, 0:W])
                nc.scalar.copy(ob[:, NVALID : 2 * NVALID], ps_b[:, :, 0:W])
                nc.sync.dma_start(
                    out=out[0, 0:128, s * ROWS : (s + 1) * ROWS, :],
                    in_=ob[:, 0 : NVALID],
                )
                nc.sync.dma_start(
                    out=out[0, 128:256, s * ROWS : (s + 1) * ROWS, :],
                    in_=ob[:, NVALID : 2 * NVALID],
                )
            xs, xq = xs_next, xq_next
            for n in range(1, NPC):
                for oc in range(OCC):
                    for s in range(0, NT, 2):
                        if n + 1 < NPC and oc == 1 and s == 0:
                            # stage the next image while this one is computing
                            xs_next = xsp.tile([C_IN, H, W], f32, name=f"xs_{n+1}", tag="xs")
                            xq_next = make_xq(n + 1)
                            for r0, nr in CH_STEADY:
                                load_chunk(xs_next, xq_next, n + 1, r0, nr)
                        s2 = s + 1 < NT
                        ps_a = psp.tile(
                            [128, ROWS, WP_], f32, tag="ps", name=f"ps_{n}_{oc}_{s}", bufs=7
                        )
                        matmul_group(xq, wt, oc, s, ps_a)
                        if s2:
                            ps_b = psp.tile(
                                [128, ROWS, WP_], f32, tag="ps", name=f"ps_{n}_{oc}_{s+1}", bufs=7
                            )
                            matmul_group(xq, wt, oc, s + 1, ps_b)
                        last = n == NPC - 1 and oc == OCC - 1 and s + 2 >= NT
                        nrows = (2 if s2 else 1) * ROWS
                        ob = obp.tile([128, nrows * W], f32)
                        if not last:
                            nc.scalar.copy(ob[:, 0 : NVALID], ps_a[:, :, 0:W])
                            if s2:
                                nc.scalar.copy(ob[:, NVALID : 2 * NVALID], ps_b[:, :, 0:W])
                            nc.sync.dma_start(
                                out=out[n, oc * 128 : (oc + 1) * 128, s * ROWS : s * ROWS + nrows, :],
                                in_=ob[:],
                            )
                        else:
                            # final (singleton) tile: split evac across
                            # ACT+DVE and the store across two DMA queues
                            hr = ROWS // 2
                            nc.scalar.copy(ob[:, 0 : NVALID // 2], ps_a[:, 0:hr, 0:W])
                            nc.sync.dma_start(
                                out=out[n, oc * 128 :, s * ROWS : s * ROWS + hr, :],
                                in_=ob[:, 0 : NVALID // 2],
                            )
                            nc.vector.tensor_copy(
                                ob[:, NVALID // 2 : NVALID], ps_a[:, hr:ROWS, 0:W]
                            )
                            nc.scalar.dma_start(
                                out=out[n, oc * 128 :, s * ROWS + hr : (s + 1) * ROWS, :],
                                in_=ob[:, NVALID // 2 : NVALID],
                            )
                if n + 1 < NPC:
                    xs, xq = xs_next, xq_next
    _split_ctrl_waits(nc)
    return nc


_NC_CACHE = None


def _ensure_axon_hooks_stub():
    """bass_utils imports antenv.axon_hooks when tracing is requested (e.g. a
    BASS_TRACE env var); the agent image's antenv lacks that module. Provide a
    no-op hook module so tracing degrades gracefully instead of crashing."""
    try:
        import antenv.axon_hooks  # noqa: F401
    except ImportError:
        import types

        mod = types.ModuleType("antenv.axon_hooks")
        mod.get_axon_ntff_profile_hook = lambda: None
        mod.set_axon_ntff_profile_hook = lambda h: None
        sys.modules["antenv.axon_hooks"] = mod


def _quantize(weight):
    """Exact replica of the reference's ternary quantization, in numpy f32."""
    t = np.float32(THRESHOLD)
    nw = (weight / np.max(np.abs(weight))).astype(np.float32)
    mask = np.where((nw > -t) & (nw <= t), np.float32(0.0), nw)
    mask = np.where(mask > t, np.float32(1.0), mask)
    mask = np.where(mask < -t, np.float32(-1.0), mask)
    qw = np.where(mask == np.float32(-1.0), np.float32(-1.0), mask)
    return qw.astype(np.float32)


def _pack_w8(weight, Wn_val):
    """(C_OUT, C_IN, 3, 3) f32 -> [C_IN, 7, 2, C_OUT] fp8 tap schedule."""
    import ml_dtypes

    qw = _quantize(weight)
    qw = np.where(qw == np.float32(-1.0), Wn_val, qw).astype(np.float32)
    # tap t=(kh,kw): lhsT[ci, co] = qw[co, ci, kh, kw]
    taps = qw.transpose(1, 2, 3, 0)  # (C_IN, 3, 3, C_OUT)
    w8 = np.zeros((C_IN, NW, 2, C_OUT), np.float32)
    for ci, (kh, kw) in enumerate(CORR):
        w8[:, ci, 0, :] = taps[:, kh, kw, :]
        w8[:, ci, 1, :] = taps[:, kh, kw, :]
    for pi, ((kh1, kw1), (kh2, kw2)) in enumerate(PAIRS):
        w8[:, len(CORR) + pi, 0, :] = taps[:, kh1, kw1, :]
        w8[:, len(CORR) + pi, 1, :] = taps[:, kh2, kw2, :]
    # -> [C_IN, NW, OCC, 2, 128]: k-tile pairs contiguous per oc chunk
    w8 = w8.reshape(C_IN, NW, 2, OCC, 128).transpose(0, 1, 3, 2, 4)
    return np.ascontiguousarray(w8).astype(ml_dtypes.float8_e4m3)


def kernel(X, weight, Wp, Wn):
    global _NC_CACHE
    X = np.ascontiguousarray(np.asarray(X, dtype=np.float32))
    weight = np.asarray(weight, dtype=np.float32)
    Wn_val = np.float32(np.asarray(Wn).reshape(-1)[0])

    w8 = _pack_w8(weight, Wn_val)

    _ensure_axon_hooks_stub()
    if _NC_CACHE is None:
        _NC_CACHE = _build_nc()
    nc = _NC_CACHE

    in_maps = [
        {"X": X[c * NPC : (c + 1) * NPC], "W8": w8} for c in range(N_CORES)
    ]
    res = run_bass_kernel_spmd(nc, in_maps, core_ids=list(range(N_CORES)))
    return np.concatenate([res.results[c]["OUT"] for c in range(N_CORES)], axis=0)


# revision 21
# speedup vs baseline: 1.0175x; 1.0175x over previous
"""Ternary-quantized 3x3 conv (stride 1, pad 1) on 8 trn2 NeuronCores.

Full inputs: X (32,128,56,56) f32, weight (256,128,3,3) f32, Wp/Wn (1,) f32.
Output: (32,256,56,56) f32.

Strategy: data-parallel over batch (4 images per core). Ternary weights are
exact in fp8e4; X is split into hi = fp8(x) and lo = fp8(x - hi). Per-core
implicit GEMM with fp8 DoubleRow matmuls (2 contraction k-tiles per
instruction at the fp16 streaming rate):
  - 5 "cross" taps (0,1),(1,0),(1,1),(1,2),(2,1): one DR matmul per tap with
    k-tiles (hi, lo) -> numerically exact to ~1e-3.
  - 4 corner taps: 2 DR matmuls, each pairing two taps at column offset +2 on
    the hi plane only (fp8-only error on 4/9 taps; measured rel err 1.7e-2).
7 matmuls per (image, oc-chunk, spatial tile) instead of 9 -> 1.29x on the
PE-bound phase. rhs windows are 464 contiguous elements (8 padded 58-wide
rows); the 2 junk columns per row are dropped during PSUM evacuation.
"""

import sys

sys.path.insert(0, "/opt/trn_rl_repo")

import numpy as np

import bass_rust
import concourse.bass as bass
import concourse.mybir as mybir
from concourse.ap import AP
from concourse.tile import TileContext
from concourse.bass_utils import run_bass_kernel_spmd

B, C_IN, C_OUT, KS, H, W = 32, 128, 256, 3, 56, 56
THRESHOLD = 0.05
N_CORES = 8
NPC = B // N_CORES  # images per core
WP_ = W + 2  # padded width (58)
HPAD = H + 3  # padded plane rows incl 1 slack row (59)
PLANE = HPAD * WP_  # k-tile stride between hi and lo planes (3422)
ROWS = 8  # output rows per spatial tile
NT = H // ROWS  # spatial tiles per image (7)
NFREE = ROWS * WP_  # matmul free dim incl junk cols (464)
NVALID = ROWS * W  # valid output elements per tile (448)
OCC = C_OUT // 128  # output channel chunks (2)

# tap schedule: corrected (hi+lo) taps and hi-only pairs
CORR = [(0, 1), (1, 0), (1, 1), (1, 2), (2, 1)]
PAIRS = [((0, 0), (0, 2)), ((2, 0), (2, 2))]  # delta = 2 columns
NW = len(CORR) + len(PAIRS)  # weight slots (7)

# walrus codegen in this container has tight per-instruction sync-wait
# encoding limits (DMA_DIRECT2D: 1, CTRL/Drain: <=2). Hoist excess waits onto
# preceding nop instructions on the same engine (safe: every non-Pool engine
# sequencer is a single strict-FIFO stream).
_MAX_WAITS = {
    "InstDMACopy": 1,
    "InstDrain": 1,
    "InstNop": 1,
    "InstNoOp": 1,
    "InstEventSemaphore": 1,
    "InstSemClear": 1,
}
_DEFAULT_MAX_WAITS = 1


def _split_ctrl_waits(nc, max_waits=None):
    for bbw in nc.main_func.blocks:
        il = bbw.instructions
        i = 0
        while i < len(il):
            ins = il[i]
            si = ins.sync_info
            if si is None or not si.on_wait:
                i += 1
                continue
            limit = _MAX_WAITS.get(type(ins).__name__, _DEFAULT_MAX_WAITS)
            if len(si.on_wait) > limit and str(ins.engine) != "EngineType.Pool":
                max_waits = limit
                waits = list(si.on_wait)
                keep, extra = waits[:max_waits], waits[max_waits:]
                new_insts = []
                for s in range(0, len(extra), max_waits):
                    chunk = extra[s : s + max_waits]
                    nop_ins = nc.engines[ins.engine].nop(nofuse=True).ins
                    for b2 in nc.main_func.blocks:
                        if b2.instructions and b2.instructions[-1] is nop_ins:
                            b2.instructions.pop()
                            break
                    nop_ins.sync_info = bass_rust.SyncInfo(
                        on_wait=chunk, on_update=[]
                    )
                    new_insts.append(nop_ins)
                si.on_wait = keep
                for k, nop_ins in enumerate(new_insts):
                    il.insert(i + k, nop_ins)
                i += len(new_insts)
            i += 1


def _build_nc():
    f32, f16, f8 = mybir.dt.float32, mybir.dt.float16, mybir.dt.float8e4
    nc = bass.Bass()
    x_in = nc.dram_tensor("X", [NPC, C_IN, H, W], f32, kind="ExternalInput")
    # k-tile pairs contiguous per oc chunk so LDWEIGHTS reads 256B strings
    w_in = nc.dram_tensor("W8", [C_IN, NW, OCC, 2, 128], f8, kind="ExternalInput")
    out = nc.dram_tensor("OUT", [NPC, C_OUT, H, W], f32, kind="ExternalOutput")

    DR = mybir.MatmulPerfMode.DoubleRow

    with TileContext(nc) as tc:
        with (
            tc.tile_pool(name="wp", bufs=1) as wp,
            tc.tile_pool(name="xs", bufs=3) as xsp,
            tc.tile_pool(name="xq", bufs=3) as xqp,
            tc.tile_pool(name="ps", bufs=8, space="PSUM") as psp,
            tc.tile_pool(name="ob", bufs=8) as obp,
        ):
            wt = wp.tile([C_IN, NW, OCC, 2, 128], f8)

            # PE warm-up: dummy matmuls keep TensorE busy through the
            # input-load phase so HAM is ramping toward K=8/8 (2.4 GHz) when
            # the real matmuls start.
            warm_sb = wp.tile([C_IN, 384], f16, name="warm_sb", tag="warm_sb")
            nc.vector.memset(warm_sb[:], 0.0)
            warm_ps = psp.tile([128, 256], f32, name="warm_ps", tag="warm", bufs=1)

            def warm(k):
                for _ in range(k):
                    nc.tensor.matmul(
                        warm_ps[:], warm_sb[:, 0:128], warm_sb[:, 128:384],
                        start=True, stop=True,
                    )

            warm(8)

            def make_xq(n, borders=True):
                # [hi/lo plane, 59 rows (58 + slack), 58 cols] fp8
                xq = xqp.tile([C_IN, 2, HPAD, WP_], f8, name=f"xq_{n}", tag="xq")
                if borders:
                    add_borders(xq)
                return xq

            def add_borders(xq):
                for k in range(2):
                    # top border row, bottom border + slack rows, side columns
                    nc.vector.memset(xq[:, k, 0, :], 0.0)
                    nc.vector.memset(xq[:, k, H + 1 : HPAD, :], 0.0)
                    nc.vector.memset(xq[:, k, 1 : H + 1, 0], 0.0)
                    nc.vector.memset(xq[:, k, 1 : H + 1, WP_ - 1], 0.0)

            def load_chunk(xs, xq, n, r0, nrows):
                nc.sync.dma_start(
                    out=xs[:, r0 : r0 + nrows, :], in_=x_in[n, :, r0 : r0 + nrows, :]
                )
                hi = xq[:, 0, r0 + 1 : r0 + nrows + 1, 1 : WP_ - 1]
                nc.vector.tensor_copy(hi, xs[:, r0 : r0 + nrows, :])
                nc.vector.scalar_tensor_tensor(
                    out=xq[:, 1, r0 + 1 : r0 + nrows + 1, 1 : WP_ - 1],
                    in0=xs[:, r0 : r0 + nrows, :],
                    scalar=0.0,
                    in1=hi,
                    op0=mybir.AluOpType.bypass,
                    op1=mybir.AluOpType.subtract,
                )

            CH_STEADY = [(0, 14), (14, 14), (28, 14), (42, 14)]
            # image 0 uses small chunks so the first groups unblock quickly
            CH_FIRST = [(7 * k, 7) for k in range(8)]

            xs0 = xsp.tile([C_IN, H, W], f32, name="xs_0", tag="xs")
            xq0 = make_xq(0, borders=False)
            # weights ride the idle scalar DMA queue so the first X chunk's
            # sync-queue issue isn't delayed behind the 460KB weight DMA
            nc.scalar.dma_start(out=wt[:], in_=w_in[:])
            # first two chunks' casts go ahead of the border memsets on the
            # DVE queue: the first matmul group needs chunks 0-1 AND borders,
            # and the casts are on the DMA-completion critical path
            for r0, nr in CH_FIRST[:2]:
                load_chunk(xs0, xq0, 0, r0, nr)
            add_borders(xq0)
            for r0, nr in CH_FIRST[2:]:
                load_chunk(xs0, xq0, 0, r0, nr)

            def pair_rhs(xq, s, kh):
                # [part][(2 cols, 2)][(1, 464)] at hi-plane offset (s*8+kh)*58
                a = xq[:]
                return AP(
                    tensor=a.tensor,
                    offset=a.offset + (s * ROWS + kh) * WP_,
                    ap=[[2 * PLANE, C_IN], [2, 2], [1, NFREE]],
                )

            def corr_rhs(xq, s, kh, kw):
                a = xq[:]
                return AP(
                    tensor=a.tensor,
                    offset=a.offset + (s * ROWS + kh) * WP_ + kw,
                    ap=[[2 * PLANE, C_IN], [PLANE, 2], [1, NFREE]],
                )

            def matmul_group(xq, wtile, oc, s, ps):
                i = 0
                for ci, (kh, kw) in enumerate(CORR):
                    nc.tensor.matmul(
                        ps[:],
                        wtile[:, ci, oc, :, :],
                        corr_rhs(xq, s, kh, kw),
                        start=(i == 0),
                        stop=(i == NW - 1),
                        perf_mode=DR,
                    )
                    i += 1
                for pi, ((kh, kw1), _t2) in enumerate(PAIRS):
                    nc.tensor.matmul(
                        ps[:],
                        wtile[:, len(CORR) + pi, oc, :, :],
                        pair_rhs(xq, s, kh),
                        start=(i == 0),
                        stop=(i == NW - 1),
                        perf_mode=DR,
                    )
                    i += 1

            xs, xq = xs0, xq0
            # image 0: s-major across both oc chunks so early groups consume
            # input rows at half the rate the cast chain produces them
            n = 0
            for s in range(NT):
                if s == 2:
                    xs_next = xsp.tile([C_IN, H, W], f32, name="xs_1", tag="xs")
                    xq_next = make_xq(1)
                    for r0, nr in CH_STEADY:
                        load_chunk(xs_next, xq_next, 1, r0, nr)
                ps_a = psp.tile([128, ROWS, WP_], f32, tag="ps", name=f"ps_0a_{s}", bufs=7)
                matmul_group(xq, wt, 0, s, ps_a)
                if s <= 2:
                    warm(2)  # bridge input-chain stalls; keep HAM ramping
                ps_b = psp.tile([128, ROWS, WP_], f32, tag="ps", name=f"ps_0b_{s}", bufs=7)
                matmul_group(xq, wt, 1, s, ps_b)
                ob = obp.tile([128, 2 * NVALID], f32)
                nc.scalar.copy(ob[:, 0 : NVALID], ps_a[:, :, 0:W])
                nc.scalar.copy(ob[:, NVALID : 2 * NVALID], ps_b[:, :, 0:W])
                nc.sync.dma_start(
                    out=out[0, 0:128, s * ROWS : (s + 1) * ROWS, :],
                    in_=ob[:, 0 : NVALID],
                )
                nc.sync.dma_start(
                    out=out[0, 128:256, s * ROWS : (s + 1) * ROWS, :],
                    in_=ob[:, NVALID : 2 * NVALID],
                )
            xs, xq = xs_next, xq_next
            for n in range(1, NPC):
                for oc in range(OCC):
                    for s in range(0, NT, 2):
                        if n + 1 < NPC and oc == 1 and s == 0:
                            # stage the next image while this one is computing
                            xs_next = xsp.tile([C_IN, H, W], f32, name=f"xs_{n+1}", tag="xs")
                            xq_next = make_xq(n + 1)
                            for r0, nr in CH_STEADY:
                                load_chunk(xs_next, xq_next, n + 1, r0, nr)
                        s2 = s + 1 < NT
                        ps_a = psp.tile(
                            [128, ROWS, WP_], f32, tag="ps", name=f"ps_{n}_{oc}_{s}", bufs=7
                        )
                        matmul_group(xq, wt, oc, s, ps_a)
                        if s2:
                            ps_b = psp.tile(
                                [128, ROWS, WP_], f32, tag="ps", name=f"ps_{n}_{oc}_{s+1}", bufs=7
                            )
                            matmul_group(xq, wt, oc, s + 1, ps_b)
                        last = n == NPC - 1 and oc == OCC - 1 and s + 2 >= NT
                        nrows = (2 if s2 else 1) * ROWS
                        ob = obp.tile([128, nrows * W], f32)
                        if not last:
                            nc.scalar.copy(ob[:, 0 : NVALID], ps_a[:, :, 0:W])
                            if s2:
                                nc.scalar.copy(ob[:, NVALID : 2 * NVALID], ps_b[:, :, 0:W])
                            nc.sync.dma_start(
                                out=out[n, oc * 128 : (oc + 1) * 128, s * ROWS : s * ROWS + nrows, :],
                                in_=ob[:],
                            )
                        else:
                            # final (singleton) tile: split evac across
                            # ACT+DVE and the store across two DMA queues
                            hr = ROWS // 2
                            nc.scalar.copy(ob[:, 0 : NVALID // 2], ps_a[:, 0:hr, 0:W])
                            nc.sync.dma_start(
                                out=out[n, oc * 128 :, s * ROWS : s * ROWS + hr, :],
                                in_=ob[:, 0 : NVALID // 2],
                            )
                            nc.vector.tensor_copy(
                                ob[:, NVALID // 2 : NVALID], ps_a[:, hr:ROWS, 0:W]
                            )
                            nc.scalar.dma_start(
                                out=out[n, oc * 128 :, s * ROWS + hr : (s + 1) * ROWS, :],
                                in_=ob[:, NVALID // 2 : NVALID],
                            )
                if n + 1 < NPC:
                    xs, xq = xs_next, xq_next
    _split_ctrl_waits(nc)
    return nc


_NC_CACHE = None


def _ensure_axon_hooks_stub():
    """bass_utils imports antenv.axon_hooks when tracing is requested (e.g. a
    BASS_TRACE env var); the agent image's antenv lacks that module. Provide a
    no-op hook module so tracing degrades gracefully instead of crashing."""
    try:
        import antenv.axon_hooks  # noqa: F401
    except ImportError:
        import types

        mod = types.ModuleType("antenv.axon_hooks")
        mod.get_axon_ntff_profile_hook = lambda: None
        mod.set_axon_ntff_profile_hook = lambda h: None
        sys.modules["antenv.axon_hooks"] = mod


def _quantize(weight):
    """Exact replica of the reference's ternary quantization, in numpy f32."""
    t = np.float32(THRESHOLD)
    nw = (weight / np.max(np.abs(weight))).astype(np.float32)
    mask = np.where((nw > -t) & (nw <= t), np.float32(0.0), nw)
    mask = np.where(mask > t, np.float32(1.0), mask)
    mask = np.where(mask < -t, np.float32(-1.0), mask)
    qw = np.where(mask == np.float32(-1.0), np.float32(-1.0), mask)
    return qw.astype(np.float32)


def _pack_w8(weight, Wn_val):
    """(C_OUT, C_IN, 3, 3) f32 -> [C_IN, 7, 2, C_OUT] fp8 tap schedule."""
    import ml_dtypes

    qw = _quantize(weight)
    qw = np.where(qw == np.float32(-1.0), Wn_val, qw).astype(np.float32)
    # tap t=(kh,kw): lhsT[ci, co] = qw[co, ci, kh, kw]
    taps = qw.transpose(1, 2, 3, 0)  # (C_IN, 3, 3, C_OUT)
    w8 = np.zeros((C_IN, NW, 2, C_OUT), np.float32)
    for ci, (kh, kw) in enumerate(CORR):
        w8[:, ci, 0, :] = taps[:, kh, kw, :]
        w8[:, ci, 1, :] = taps[:, kh, kw, :]
    for pi, ((kh1, kw1), (kh2, kw2)) in enumerate(PAIRS):
        w8[:, len(CORR) + pi, 0, :] = taps[:, kh1, kw1, :]
        w8[:, len(CORR) + pi, 1, :] = taps[:, kh2, kw2, :]
    # -> [C_IN, NW, OCC, 2, 128]: k-tile pairs contiguous per oc chunk
    w8 = w8.reshape(C_IN, NW, 2, OCC, 128).transpose(0, 1, 3, 2, 4)
    return np.ascontiguousarray(w8).astype(ml_dtypes.float8_e4m3)


def kernel(X, weight, Wp, Wn):
    global _NC_CACHE
    X = np.ascontiguousarray(np.asarray(X, dtype=np.float32))
    weight = np.asarray(weight, dtype=np.float32)
    Wn_val = np.float32(np.asarray(Wn).reshape(-1)[0])

    w8 = _pack_w8(weight, Wn_val)

    _ensure_axon_hooks_stub()
    if _NC_CACHE is None:
        _NC_CACHE = _build_nc()
    nc = _NC_CACHE

    in_maps = [
        {"X": X[c * NPC : (c + 1) * NPC], "W8": w8} for c in range(N_CORES)
    ]
    res = run_bass_kernel_spmd(nc, in_maps, core_ids=list(range(N_CORES)))
    return np.concatenate([res.results[c]["OUT"] for c in range(N_CORES)], axis=0)


# revision 22
# speedup vs baseline: 1.0192x; 1.0016x over previous
"""Ternary-quantized 3x3 conv (stride 1, pad 1) on 8 trn2 NeuronCores.

Full inputs: X (32,128,56,56) f32, weight (256,128,3,3) f32, Wp/Wn (1,) f32.
Output: (32,256,56,56) f32.

Strategy: data-parallel over batch (4 images per core). Ternary weights are
exact in fp8e4; X is split into hi = fp8(x) and lo = fp8(x - hi). Per-core
implicit GEMM with fp8 DoubleRow matmuls (2 contraction k-tiles per
instruction at the fp16 streaming rate):
  - 5 "cross" taps (0,1),(1,0),(1,1),(1,2),(2,1): one DR matmul per tap with
    k-tiles (hi, lo) -> numerically exact to ~1e-3.
  - 4 corner taps: 2 DR matmuls, each pairing two taps at column offset +2 on
    the hi plane only (fp8-only error on 4/9 taps; measured rel err 1.7e-2).
7 matmuls per (image, oc-chunk, spatial tile) instead of 9 -> 1.29x on the
PE-bound phase. rhs windows are 464 contiguous elements (8 padded 58-wide
rows); the 2 junk columns per row are dropped during PSUM evacuation.
"""

import sys

sys.path.insert(0, "/opt/trn_rl_repo")

import numpy as np

import bass_rust
import concourse.bass as bass
import concourse.mybir as mybir
from concourse.ap import AP
from concourse.tile import TileContext
from concourse.bass_utils import run_bass_kernel_spmd

B, C_IN, C_OUT, KS, H, W = 32, 128, 256, 3, 56, 56
THRESHOLD = 0.05
N_CORES = 8
NPC = B // N_CORES  # images per core
WP_ = W + 2  # padded width (58)
HPAD = H + 3  # padded plane rows incl 1 slack row (59)
PLANE = HPAD * WP_  # k-tile stride between hi and lo planes (3422)
ROWS = 8  # output rows per spatial tile
NT = H // ROWS  # spatial tiles per image (7)
NFREE = ROWS * WP_  # matmul free dim incl junk cols (464)
NVALID = ROWS * W  # valid output elements per tile (448)
OCC = C_OUT // 128  # output channel chunks (2)

# tap schedule: corrected (hi+lo) taps and hi-only pairs
CORR = [(0, 1), (1, 0), (1, 1), (1, 2), (2, 1)]
PAIRS = [((0, 0), (0, 2)), ((2, 0), (2, 2))]  # delta = 2 columns
NW = len(CORR) + len(PAIRS)  # weight slots (7)

# walrus codegen in this container has tight per-instruction sync-wait
# encoding limits (DMA_DIRECT2D: 1, CTRL/Drain: <=2). Hoist excess waits onto
# preceding nop instructions on the same engine (safe: every non-Pool engine
# sequencer is a single strict-FIFO stream).
_MAX_WAITS = {
    "InstDMACopy": 1,
    "InstDrain": 1,
    "InstNop": 1,
    "InstNoOp": 1,
    "InstEventSemaphore": 1,
    "InstSemClear": 1,
}
_DEFAULT_MAX_WAITS = 1


def _split_ctrl_waits(nc, max_waits=None):
    for bbw in nc.main_func.blocks:
        il = bbw.instructions
        i = 0
        while i < len(il):
            ins = il[i]
            si = ins.sync_info
            if si is None or not si.on_wait:
                i += 1
                continue
            limit = _MAX_WAITS.get(type(ins).__name__, _DEFAULT_MAX_WAITS)
            if len(si.on_wait) > limit and str(ins.engine) != "EngineType.Pool":
                max_waits = limit
                waits = list(si.on_wait)
                keep, extra = waits[:max_waits], waits[max_waits:]
                new_insts = []
                for s in range(0, len(extra), max_waits):
                    chunk = extra[s : s + max_waits]
                    nop_ins = nc.engines[ins.engine].nop(nofuse=True).ins
                    for b2 in nc.main_func.blocks:
                        if b2.instructions and b2.instructions[-1] is nop_ins:
                            b2.instructions.pop()
                            break
                    nop_ins.sync_info = bass_rust.SyncInfo(
                        on_wait=chunk, on_update=[]
                    )
                    new_insts.append(nop_ins)
                si.on_wait = keep
                for k, nop_ins in enumerate(new_insts):
                    il.insert(i + k, nop_ins)
                i += len(new_insts)
            i += 1


def _build_nc():
    f32, f16, f8 = mybir.dt.float32, mybir.dt.float16, mybir.dt.float8e4
    nc = bass.Bass()
    x_in = nc.dram_tensor("X", [NPC, C_IN, H, W], f32, kind="ExternalInput")
    # k-tile pairs contiguous per oc chunk so LDWEIGHTS reads 256B strings
    w_in = nc.dram_tensor("W8", [C_IN, NW, OCC, 2, 128], f8, kind="ExternalInput")
    out = nc.dram_tensor("OUT", [NPC, C_OUT, H, W], f32, kind="ExternalOutput")

    DR = mybir.MatmulPerfMode.DoubleRow

    with TileContext(nc) as tc:
        with (
            tc.tile_pool(name="wp", bufs=1) as wp,
            tc.tile_pool(name="xs", bufs=3) as xsp,
            tc.tile_pool(name="xq", bufs=3) as xqp,
            tc.tile_pool(name="ps", bufs=8, space="PSUM") as psp,
            tc.tile_pool(name="ob", bufs=8) as obp,
        ):
            wt = wp.tile([C_IN, NW, OCC, 2, 128], f8)

            # PE warm-up: dummy matmuls keep TensorE busy through the
            # input-load phase so HAM is ramping toward K=8/8 (2.4 GHz) when
            # the real matmuls start.
            warm_sb = wp.tile([C_IN, 384], f16, name="warm_sb", tag="warm_sb")
            nc.vector.memset(warm_sb[:], 0.0)
            warm_ps = psp.tile([128, 256], f32, name="warm_ps", tag="warm", bufs=1)

            def warm(k):
                for _ in range(k):
                    nc.tensor.matmul(
                        warm_ps[:], warm_sb[:, 0:128], warm_sb[:, 128:384],
                        start=True, stop=True,
                    )

            warm(8)

            def make_xq(n, borders=True):
                # [hi/lo plane, 59 rows (58 + slack), 58 cols] fp8
                xq = xqp.tile([C_IN, 2, HPAD, WP_], f8, name=f"xq_{n}", tag="xq")
                if borders:
                    add_borders(xq)
                return xq

            def add_borders(xq):
                for k in range(2):
                    # top border row, bottom border + slack rows, side columns
                    nc.vector.memset(xq[:, k, 0, :], 0.0)
                    nc.vector.memset(xq[:, k, H + 1 : HPAD, :], 0.0)
                    nc.vector.memset(xq[:, k, 1 : H + 1, 0], 0.0)
                    nc.vector.memset(xq[:, k, 1 : H + 1, WP_ - 1], 0.0)

            def load_chunk(xs, xq, n, r0, nrows):
                nc.sync.dma_start(
                    out=xs[:, r0 : r0 + nrows, :], in_=x_in[n, :, r0 : r0 + nrows, :]
                )
                hi = xq[:, 0, r0 + 1 : r0 + nrows + 1, 1 : WP_ - 1]
                nc.vector.tensor_copy(hi, xs[:, r0 : r0 + nrows, :])
                nc.vector.scalar_tensor_tensor(
                    out=xq[:, 1, r0 + 1 : r0 + nrows + 1, 1 : WP_ - 1],
                    in0=xs[:, r0 : r0 + nrows, :],
                    scalar=0.0,
                    in1=hi,
                    op0=mybir.AluOpType.bypass,
                    op1=mybir.AluOpType.subtract,
                )

            CH_STEADY = [(0, 14), (14, 14), (28, 14), (42, 14)]
            # image 0 uses small chunks so the first groups unblock quickly
            CH_FIRST = [(7 * k, 7) for k in range(8)]

            xs0 = xsp.tile([C_IN, H, W], f32, name="xs_0", tag="xs")
            xq0 = make_xq(0, borders=False)
            # weights ride the idle scalar DMA queue so the first X chunk's
            # sync-queue issue isn't delayed behind the 460KB weight DMA
            nc.scalar.dma_start(out=wt[:], in_=w_in[:])
            # first two chunks' casts go ahead of the border memsets on the
            # DVE queue: the first matmul group needs chunks 0-1 AND borders,
            # and the casts are on the DMA-completion critical path
            for r0, nr in CH_FIRST[:2]:
                load_chunk(xs0, xq0, 0, r0, nr)
            add_borders(xq0)
            for r0, nr in CH_FIRST[2:]:
                load_chunk(xs0, xq0, 0, r0, nr)

            def pair_rhs(xq, s, kh):
                # [part][(2 cols, 2)][(58, 8)][(1, 56)] at hi-plane offset
                a = xq[:]
                return AP(
                    tensor=a.tensor,
                    offset=a.offset + (s * ROWS + kh) * WP_,
                    ap=[[2 * PLANE, C_IN], [2, 2], [WP_, ROWS], [1, W]],
                )

            def corr_rhs(xq, s, kh, kw):
                a = xq[:]
                return AP(
                    tensor=a.tensor,
                    offset=a.offset + (s * ROWS + kh) * WP_ + kw,
                    ap=[[2 * PLANE, C_IN], [PLANE, 2], [WP_, ROWS], [1, W]],
                )

            def matmul_group(xq, wtile, oc, s, ps):
                i = 0
                for ci, (kh, kw) in enumerate(CORR):
                    nc.tensor.matmul(
                        ps[:],
                        wtile[:, ci, oc, :, :],
                        corr_rhs(xq, s, kh, kw),
                        start=(i == 0),
                        stop=(i == NW - 1),
                        perf_mode=DR,
                    )
                    i += 1
                for pi, ((kh, kw1), _t2) in enumerate(PAIRS):
                    nc.tensor.matmul(
                        ps[:],
                        wtile[:, len(CORR) + pi, oc, :, :],
                        pair_rhs(xq, s, kh),
                        start=(i == 0),
                        stop=(i == NW - 1),
                        perf_mode=DR,
                    )
                    i += 1

            xs, xq = xs0, xq0
            # image 0: s-major across both oc chunks so early groups consume
            # input rows at half the rate the cast chain produces them
            n = 0
            for s in range(NT):
                if s == 2:
                    xs_next = xsp.tile([C_IN, H, W], f32, name="xs_1", tag="xs")
                    xq_next = make_xq(1)
                    for r0, nr in CH_STEADY:
                        load_chunk(xs_next, xq_next, 1, r0, nr)
                ps_a = psp.tile([128, ROWS, W], f32, tag="ps", name=f"ps_0a_{s}", bufs=7)
                matmul_group(xq, wt, 0, s, ps_a)
                if s <= 2:
                    warm(2)  # bridge input-chain stalls; keep HAM ramping
                ps_b = psp.tile([128, ROWS, W], f32, tag="ps", name=f"ps_0b_{s}", bufs=7)
                matmul_group(xq, wt, 1, s, ps_b)
                ob = obp.tile([128, 2 * NVALID], f32)
                nc.scalar.copy(ob[:, 0 : NVALID], ps_a[:, :, 0:W])
                nc.scalar.copy(ob[:, NVALID : 2 * NVALID], ps_b[:, :, 0:W])
                nc.sync.dma_start(
                    out=out[0, 0:128, s * ROWS : (s + 1) * ROWS, :],
                    in_=ob[:, 0 : NVALID],
                )
                nc.sync.dma_start(
                    out=out[0, 128:256, s * ROWS : (s + 1) * ROWS, :],
                    in_=ob[:, NVALID : 2 * NVALID],
                )
            xs, xq = xs_next, xq_next
            for n in range(1, NPC):
                for oc in range(OCC):
                    for s in range(0, NT, 2):
                        if n + 1 < NPC and oc == 1 and s == 0:
                            # stage the next image while this one is computing
                            xs_next = xsp.tile([C_IN, H, W], f32, name=f"xs_{n+1}", tag="xs")
                            xq_next = make_xq(n + 1)
                            for r0, nr in CH_STEADY:
                                load_chunk(xs_next, xq_next, n + 1, r0, nr)
                        s2 = s + 1 < NT
                        ps_a = psp.tile(
                            [128, ROWS, W], f32, tag="ps", name=f"ps_{n}_{oc}_{s}", bufs=7
                        )
                        matmul_group(xq, wt, oc, s, ps_a)
                        if s2:
                            ps_b = psp.tile(
                                [128, ROWS, W], f32, tag="ps", name=f"ps_{n}_{oc}_{s+1}", bufs=7
                            )
                            matmul_group(xq, wt, oc, s + 1, ps_b)
                        last = n == NPC - 1 and oc == OCC - 1 and s + 2 >= NT
                        nrows = (2 if s2 else 1) * ROWS
                        ob = obp.tile([128, nrows * W], f32)
                        if not last:
                            nc.scalar.copy(ob[:, 0 : NVALID], ps_a[:, :, 0:W])
                            if s2:
                                nc.scalar.copy(ob[:, NVALID : 2 * NVALID], ps_b[:, :, 0:W])
                            nc.sync.dma_start(
                                out=out[n, oc * 128 : (oc + 1) * 128, s * ROWS : s * ROWS + nrows, :],
                                in_=ob[:],
                            )
                        else:
                            # final (singleton) tile: split evac across
                            # ACT+DVE and the store across two DMA queues
                            hr = ROWS // 2
                            nc.scalar.copy(ob[:, 0 : NVALID // 2], ps_a[:, 0:hr, 0:W])
                            nc.sync.dma_start(
                                out=out[n, oc * 128 :, s * ROWS : s * ROWS + hr, :],
                                in_=ob[:, 0 : NVALID // 2],
                            )
                            nc.vector.tensor_copy(
                                ob[:, NVALID // 2 : NVALID], ps_a[:, hr:ROWS, 0:W]
                            )
                            nc.scalar.dma_start(
                                out=out[n, oc * 128 :, s * ROWS + hr : (s + 1) * ROWS, :],
                                in_=ob[:, NVALID // 2 : NVALID],
                            )
                if n + 1 < NPC:
                    xs, xq = xs_next, xq_next
    _split_ctrl_waits(nc)
    return nc


_NC_CACHE = None


def _ensure_axon_hooks_stub():
    """bass_utils imports antenv.axon_hooks when tracing is requested (e.g. a
    BASS_TRACE env var); the agent image's antenv lacks that module. Provide a
    no-op hook module so tracing degrades gracefully instead of crashing."""
    try:
        import antenv.axon_hooks  # noqa: F401
    except ImportError:
        import types

        mod = types.ModuleType("antenv.axon_hooks")
        mod.get_axon_ntff_profile_hook = lambda: None
        mod.set_axon_ntff_profile_hook = lambda h: None
        sys.modules["antenv.axon_hooks"] = mod


def _quantize(weight):
    """Exact replica of the reference's ternary quantization, in numpy f32."""
    t = np.float32(THRESHOLD)
    nw = (weight / np.max(np.abs(weight))).astype(np.float32)
    mask = np.where((nw > -t) & (nw <= t), np.float32(0.0), nw)
    mask = np.where(mask > t, np.float32(1.0), mask)
    mask = np.where(mask < -t, np.float32(-1.0), mask)
    qw = np.where(mask == np.float32(-1.0), np.float32(-1.0), mask)
    return qw.astype(np.float32)


def _pack_w8(weight, Wn_val):
    """(C_OUT, C_IN, 3, 3) f32 -> [C_IN, 7, 2, C_OUT] fp8 tap schedule."""
    import ml_dtypes

    qw = _quantize(weight)
    qw = np.where(qw == np.float32(-1.0), Wn_val, qw).astype(np.float32)
    # tap t=(kh,kw): lhsT[ci, co] = qw[co, ci, kh, kw]
    taps = qw.transpose(1, 2, 3, 0)  # (C_IN, 3, 3, C_OUT)
    w8 = np.zeros((C_IN, NW, 2, C_OUT), np.float32)
    for ci, (kh, kw) in enumerate(CORR):
        w8[:, ci, 0, :] = taps[:, kh, kw, :]
        w8[:, ci, 1, :] = taps[:, kh, kw, :]
    for pi, ((kh1, kw1), (kh2, kw2)) in enumerate(PAIRS):
        w8[:, len(CORR) + pi, 0, :] = taps[:, kh1, kw1, :]
        w8[:, len(CORR) + pi, 1, :] = taps[:, kh2, kw2, :]
    # -> [C_IN, NW, OCC, 2, 128]: k-tile pairs contiguous per oc chunk
    w8 = w8.reshape(C_IN, NW, 2, OCC, 128).transpose(0, 1, 3, 2, 4)
    return np.ascontiguousarray(w8).astype(ml_dtypes.float8_e4m3)


def kernel(X, weight, Wp, Wn):
    global _NC_CACHE
    X = np.ascontiguousarray(np.asarray(X, dtype=np.float32))
    weight = np.asarray(weight, dtype=np.float32)
    Wn_val = np.float32(np.asarray(Wn).reshape(-1)[0])

    w8 = _pack_w8(weight, Wn_val)

    _ensure_axon_hooks_stub()
    if _NC_CACHE is None:
        _NC_CACHE = _build_nc()
    nc = _NC_CACHE

    in_maps = [
        {"X": X[c * NPC : (c + 1) * NPC], "W8": w8} for c in range(N_CORES)
    ]
    res = run_bass_kernel_spmd(nc, in_maps, core_ids=list(range(N_CORES)))
    return np.concatenate([res.results[c]["OUT"] for c in range(N_CORES)], axis=0)
